# revision 4
# baseline (speedup 1.0000x reference)
"""RNN-T JointNetwork kernel for 8x Trainium2 NeuronCores.

reference:
    enc_proj = einsum('btud,jd->btuj', enc_out, W_enc) + b_enc   # (B,T,1,J)
    dec_proj = einsum('btud,jd->btuj', dec_out, W_dec) + b_dec   # (B,1,U,J)
    joint    = tanh(enc_proj + dec_proj)                         # (B,T,U,J)
    out      = einsum('btuj,vj->btuv', joint, W_out) + b_out     # (B,T,U,V)

Strategy: data-parallel over batch B=8 across the 8 cores (one b each).
Per core:
  - enc_projT [J, T] and dec_projT [J, U] via small GEMMs (weights stationary,
    host-pretransposed operands), bias_joint = b_enc+b_dec folded into dec_projT.
  - loop over 32 f-chunks (f = t*U+u, 8 t-values x 64 u = 512 f per chunk):
      jointT[j, f] = tanh(enc_projT[j,t] + dec_projT[j,u])  (DVE bcast-add + ACT tanh)
      outT[v, f]   = W_outT.T @ jointT   (f32r matmuls, PSUM accum over 5 j-tiles,
                     8 v-tiles of 128 partitions each)
      out-stage: PSUM -> SBUF f16 with the per-partition bias b_out[v] fused,
      alternating between ACT (activation Identity+bias) and DVE
      (tensor_scalar_add) so neither engine is near-critical, then contiguous
      DMA of [128v, 512f] to the f16 DRAM output out_T [V, T*U].
All matmuls use float32r (TF32-like: full-rate streaming, fp32 accumulate);
per-core NEFF time sits at the f32r tensor roofline (~275 us/application,
TimelineSim). The output lives transposed [V, T*U] so the final bias is a
per-partition scalar (free on ACT, cheap on DVE) instead of a free-dim vector
add on DVE over the whole [T*U, V] tensor (which made DVE near-critical at
~85%% busy in the previous layout). The device output is stored float16
(~2e-4 RMS quantization, vs the 2e-2 accuracy gate) to halve HBM write
traffic; kernel() converts back to float32 + transposes on the host.
build_program(repeat=N) replays N applications in one NEFF (weights loaded
once; hw_loop=True uses a hardware For_i loop so instruction count stays
constant) so test.py can measure steady-state per-application time with
dispatch overhead amortized; kernel() itself uses repeat=1.
"""

import sys

import numpy as np

if "/opt/trn_rl_repo" not in sys.path:
    sys.path.insert(0, "/opt/trn_rl_repo")

B, T, U = 8, 256, 64
D, J, V = 512, 640, 1024
P = 128
ND, NJ, NV = D // P, J // P, V // P  # 4, 5, 8
TCH = 8  # t-values per f-chunk
NCHUNK = T // TCH  # 32
FCH = TCH * U  # 512 f-positions per chunk
F = T * U  # 16384

_prog_cache = {}


def build_program(repeat=1, hw_loop=False, inner_unroll=1):
    """Build the per-core program.

    repeat > 1 replays the full computation (projections + joint + final
    GEMM + output DMA) that many times inside one NEFF, with weights loaded
    once — used by test.py to amortize per-dispatch transport overhead when
    measuring steady-state per-application HW time. kernel() uses repeat=1.
    hw_loop=True uses a hardware For_i loop for the repeats (constant
    instruction count); hw_loop=False unrolls in Python. inner_unroll
    (hw_loop only) unrolls that many applications inside the loop body to
    amortize the per-iteration all-engine barrier of For_i.
    """
    import concourse.tile as tile
    from concourse import bacc, mybir

    f32 = mybir.dt.float32
    f32r = mybir.dt.float32r
    f16 = mybir.dt.float16
    Tanh = mybir.ActivationFunctionType.Tanh
    Ident = mybir.ActivationFunctionType.Identity

    nc = bacc.Bacc("TRN2", target_bir_lowering=False, debug=False)

    enc_T = nc.dram_tensor("enc_T", [D, T], f32, kind="ExternalInput").ap()
    dec_T = nc.dram_tensor("dec_T", [D, U], f32, kind="ExternalInput").ap()
    w_enc_T = nc.dram_tensor("w_enc_T", [D, J], f32, kind="ExternalInput").ap()
    w_dec_T = nc.dram_tensor("w_dec_T", [D, J], f32, kind="ExternalInput").ap()
    w_out_T = nc.dram_tensor("w_out_T", [J, V], f32, kind="ExternalInput").ap()
    bias_j = nc.dram_tensor("bias_j", [J, 1], f32, kind="ExternalInput").ap()
    bias_v = nc.dram_tensor("bias_v", [V, 1], f32, kind="ExternalInput").ap()
    # Output is stored f16 transposed [V, T*U] (host converts back): f16
    # halves the HBM write traffic and host<->device bytes (~2e-4 RMS err,
    # vs the 2e-2 gate); the [v, f] layout makes b_out a per-partition
    # scalar so the bias-add fuses into the PSUM->SBUF drain.
    out = nc.dram_tensor("out_T", [V, F], f16, kind="ExternalOutput").ap()

    with tile.TileContext(nc) as tc:
        with (
            tc.tile_pool(name="const", bufs=1) as constp,
            tc.tile_pool(name="proj", bufs=1) as projp,
            tc.tile_pool(name="pre", bufs=6) as prep,
            tc.tile_pool(name="joint", bufs=10) as jointp,
            tc.tile_pool(name="osb", bufs=8) as osbp,
            tc.tile_pool(name="ps", bufs=8, space="PSUM") as psp,
        ):
            # ---- load weights / inputs (one-time) ----
            # f32r matmul operands must be written by a rounding producer
            # (BIR verifier) — stage DMA loads in f32 then round-copy to f32r.
            def load_round(shape, dram_ap, tag):
                stg = constp.tile(shape, f32, tag=f"stage_{tag}")
                nc.sync.dma_start(out=stg[:], in_=dram_ap)
                t_ = constp.tile(shape, f32r, tag=tag)
                nc.vector.tensor_copy(t_[:], stg[:])
                return t_

            w_out_sb = [
                load_round([P, V], w_out_T[jt * P : (jt + 1) * P, :], f"wout{jt}")
                for jt in range(NJ)
            ]
            enc_sb, dec_sb, wenc_sb, wdec_sb = [], [], [], []
            for dt_ in range(ND):
                sl = slice(dt_ * P, (dt_ + 1) * P)
                enc_sb.append(load_round([P, T], enc_T[sl, :], f"enc{dt_}"))
                dec_sb.append(load_round([P, U], dec_T[sl, :], f"dec{dt_}"))
                wenc_sb.append(load_round([P, J], w_enc_T[sl, :], f"wenc{dt_}"))
                wdec_sb.append(load_round([P, J], w_dec_T[sl, :], f"wdec{dt_}"))
            bj_sb = constp.tile([P, NJ], f32, tag="bj")
            nc.sync.dma_start(
                out=bj_sb[:],
                in_=bias_j.rearrange("(jt p) one -> p (jt one)", p=P),
            )
            bv_sb = constp.tile([P, NV], f32, tag="bv")
            nc.sync.dma_start(
                out=bv_sb[:],
                in_=bias_v.rearrange("(vt p) one -> p (vt one)", p=P),
            )

            # ---- repeated body: projections + joint + final GEMM ----
            if hw_loop and repeat > 1:
                assert repeat % inner_unroll == 0
                with tc.For_i(0, repeat // inner_unroll):
                    for _inner in range(inner_unroll):
                        run_body(nc, tc, projp, prep, jointp, osbp, psp,
                                 enc_sb, dec_sb, wenc_sb, wdec_sb, w_out_sb,
                                 bj_sb, bv_sb, out, f32, f32r, f16, Tanh,
                                 Ident)
            else:
                for _rep in range(repeat):
                    run_body(nc, tc, projp, prep, jointp, osbp, psp, enc_sb,
                             dec_sb, wenc_sb, wdec_sb, w_out_sb, bj_sb,
                             bv_sb, out, f32, f32r, f16, Tanh, Ident)
    nc.compile()
    return nc


def run_body(nc, tc, projp, prep, jointp, osbp, psp, enc_sb, dec_sb,
             wenc_sb, wdec_sb, w_out_sb, bj_sb, bv_sb, out,
             f32, f32r, f16, Tanh, Ident):
    P = 128
    # ---- projections: enc_projT [J, T], dec_projT [J, U] ----
    enc_proj, dec_proj = [], []
    for jt in range(NJ):
        ps = psp.tile([P, FCH], f32, tag="ps")
        for dt_ in range(ND):
            nc.tensor.matmul(
                ps[:, :T],
                lhsT=wenc_sb[dt_][:, jt * P : (jt + 1) * P],
                rhs=enc_sb[dt_][:],
                start=(dt_ == 0),
                stop=(dt_ == ND - 1),
            )
        t_ = projp.tile([P, T], f32, tag=f"encproj{jt}")
        nc.scalar.copy(t_[:], ps[:, :T])
        enc_proj.append(t_)
    for jt in range(NJ):
        ps = psp.tile([P, FCH], f32, tag="ps")
        for dt_ in range(ND):
            nc.tensor.matmul(
                ps[:, :U],
                lhsT=wdec_sb[dt_][:, jt * P : (jt + 1) * P],
                rhs=dec_sb[dt_][:],
                start=(dt_ == 0),
                stop=(dt_ == ND - 1),
            )
        t_ = projp.tile([P, U], f32, tag=f"decproj{jt}")
        nc.scalar.activation(t_[:], ps[:, :U], Ident, bias=bj_sb[:, jt : jt + 1])
        dec_proj.append(t_)

    # ---- main loop over f-chunks ----
    for ch in range(NCHUNK):
        jts = []
        for jt in range(NJ):
            pre = prep.tile([P, FCH], f32, tag="pre")
            enc_b = (
                enc_proj[jt][:, ch * TCH : (ch + 1) * TCH]
                .unsqueeze(2)
                .broadcast_to([P, TCH, U])
            )
            dec_b = dec_proj[jt][:].unsqueeze(1).broadcast_to([P, TCH, U])
            nc.vector.tensor_add(
                pre[:].rearrange("p (t u) -> p t u", t=TCH), enc_b, dec_b
            )
            jtl = jointp.tile([P, FCH], f32r, tag="joint")
            nc.scalar.activation(jtl[:], pre[:], Tanh)
            jts.append(jtl)
        for vt in range(NV):
            ps = psp.tile([P, FCH], f32, tag="ps")
            for jt in range(NJ):
                nc.tensor.matmul(
                    ps[:],
                    lhsT=w_out_sb[jt][:, vt * P : (vt + 1) * P],
                    rhs=jts[jt][:],
                    start=(jt == 0),
                    stop=(jt == NJ - 1),
                )
            o = osbp.tile([P, FCH], f16, tag="osb")
            # Fused PSUM drain + b_out[v] bias (per-partition scalar),
            # alternating ACT/DVE so neither engine becomes critical.
            if vt % 2 == 0:
                nc.scalar.activation(o[:], ps[:], Ident, bias=bv_sb[:, vt : vt + 1])
            else:
                nc.vector.tensor_scalar_add(o[:], ps[:], bv_sb[:, vt : vt + 1])
            nc.sync.dma_start(
                out=out[vt * P : (vt + 1) * P, ch * FCH : (ch + 1) * FCH],
                in_=o[:],
            )


def _get_program():
    if "nc" not in _prog_cache:
        _prog_cache["nc"] = build_program()
    return _prog_cache["nc"]


def make_in_maps(inputs):
    enc_out = np.asarray(inputs["enc_out"], dtype=np.float32)  # (B, T, 1, D)
    dec_out = np.asarray(inputs["dec_out"], dtype=np.float32)  # (B, 1, U, D)
    W_enc = np.asarray(inputs["W_enc"], dtype=np.float32)  # (J, D)
    b_enc = np.asarray(inputs["b_enc"], dtype=np.float32)
    W_dec = np.asarray(inputs["W_dec"], dtype=np.float32)
    b_dec = np.asarray(inputs["b_dec"], dtype=np.float32)
    W_out = np.asarray(inputs["W_out"], dtype=np.float32)  # (V, J)
    b_out = np.asarray(inputs["b_out"], dtype=np.float32)

    w_enc_T = np.ascontiguousarray(W_enc.T)  # [D, J]
    w_dec_T = np.ascontiguousarray(W_dec.T)  # [D, J]
    w_out_T = np.ascontiguousarray(W_out.T)  # [J, V]
    bias_j = np.ascontiguousarray((b_enc + b_dec).reshape(J, 1))
    bias_v = np.ascontiguousarray(b_out.reshape(V, 1))

    in_maps = []
    for b in range(B):
        in_maps.append(
            {
                "enc_T": np.ascontiguousarray(enc_out[b, :, 0, :].T),  # [D, T]
                "dec_T": np.ascontiguousarray(dec_out[b, 0, :, :].T),  # [D, U]
                "w_enc_T": w_enc_T,
                "w_dec_T": w_dec_T,
                "w_out_T": w_out_T,
                "bias_j": bias_j,
                "bias_v": bias_v,
            }
        )
    return in_maps


def unpack_out(arr):
    """Device out_T [V, T*U] f16 -> full-precision (T, U, V) f32."""
    return np.ascontiguousarray(
        np.asarray(arr).astype(np.float32).reshape(V, T, U).transpose(1, 2, 0)
    )


def kernel(**inputs):
    from concourse.bass_utils import run_bass_kernel_spmd

    nc = _get_program()
    in_maps = make_in_maps(inputs)
    res = run_bass_kernel_spmd(nc, in_maps, list(range(B)))
    outs = [unpack_out(res.results[i]["out_T"]) for i in range(B)]
    return np.stack(outs, axis=0)
